# revision 72
# baseline (speedup 1.0000x reference)
"""Causal self-attention (GPT-style, B=4 T=2048 C=768 H=12) on 8 trn2 cores.

Sharding: core = (batch b, head-group g), g in {0,1} covering 6 heads.
Each core: qkv projections for its 6 heads, causal flash-style attention,
partial c_proj over its 384 contraction rows; host adds the two partials
per batch plus the analytic bias row.

Key device-side structure (per core):
  x^T and w_attn ship as bf16 (halves DMA); output ships as bf16.
  QKV projections in bf16 via three 128-channel psum tiles per Q and K;
  a host-side w_attn column permutation puts heads 0-3's d-halves in
  tiles 0/1 so their Q/K evict to fp8e4 in a [32h+d%32, d-half, T] pair
  layout with partition-identity copies; heads 4-5 evict to bf16
  d-major.  Heads 0-3 run S^T = K Q^T as fp8 DoubleRow matmuls (half
  cycles/col; head 3 sits at partition base 96, which requires passing
  tile_position explicitly); heads 4-5 run plain bf16 S.
  exp on ScalarE over both heads of a pair at once ([128, 2, w] from a
  2-bank psum tile) -> P in bf16; causal diagonal masked post-exp by a
  DVE multiply.
  [V_h | 1] interleaved bf16 matmul accumulates y^T (64 partitions) and
  softmax row-sums (other 64) per (head, k-tile) into one psum bank; the
  ones blocks are memset per k-tile, V evicted with strided copies.
  Normalize: evict psum, gpsimd partition-shift of the row-sums, one
  fast reciprocal, two multiplies -> y^T fp32 (column-sliced for the
  final pair so the tail c_proj starts early).
  proj: out[t,e] = sum_f y^T[f,t] wp[f,e] in fp32r.

Scheduling: engines execute their streams in order, so PV(k-1) is issued
after S(k) -- the PE stream never blocks on exp(k) -- and cross-phase
matmul groups (next tile's QKV/V, deferred c_proj) are interleaved as
fillers into the attention loop, skewed toward the later (Act-bound)
tiles.  Startup streams (wa[ct], x[ct]) pairs on separate DGE queues so
the first QKV matmul fires as early as possible.

fp8 S on 8 of 12 heads gives ~9.7e-3 end-to-end rel err (vs 2.2e-3
all-bf16; gate 2e-2).  fp8 for QKV inputs, P, or V was measured at
3.7e-2..4.6e-2 -- rejected.
"""

from contextlib import ExitStack

import numpy as np

import concourse.bass as bass
import concourse.mybir as mybir
import concourse.tile as tile
from concourse import bacc
from concourse.masks import make_upper_triangular

AF = mybir.ActivationFunctionType
F32 = mybir.dt.float32
F32R = mybir.dt.float32r
BF16 = mybir.dt.bfloat16
F8 = mybir.dt.float8e4
DR = mybir.MatmulPerfMode.DoubleRow

C = 768          # model dim
D = 64           # head dim
HG = 6           # heads per core
NP = 3           # head pairs per core
GC = HG * D      # 384 group channels
CT = C // 128    # 6 contraction tiles
QBLK = 512       # query tile (psum bank)
KBLK = 128       # key tile (partition dim)

def _qk_perm():
    """Channel permutation (within the 384 group channels) for Q and K.
    Tile 0 holds d0-31 of heads 0-3, tile 1 d32-63 of heads 0-3, tile 2
    heads 4-5 in natural d-major order.  perm[n] = original channel
    feeding new channel n."""
    perm = np.empty(GC, dtype=np.int64)
    for n in range(GC):
        t, slot = divmod(n, 128)
        if t < 2:
            head, dd = divmod(slot, 32)
            perm[n] = head * D + 32 * t + dd
        else:
            head, dd = divmod(slot, 64)
            perm[n] = (4 + head) * D + dd
    return perm


_REGIONS = []      # (label, next_instruction_index) probes for trace analysis


def _mark(nc, label):
    _REGIONS.append((label,
                     int(nc.get_next_instruction_name().split("-")[-1])))


def build_nc(T=2048):
    NQ = T // QBLK
    NK = T // KBLK
    nc = bacc.Bacc(None)

    xt_d = nc.dram_tensor("xt", [C, T], BF16, kind="ExternalInput")
    wa_d = nc.dram_tensor("wa", [C, 3 * GC], BF16, kind="ExternalInput")
    bqk_d = nc.dram_tensor("bqk", [128, 2, 3], F32, kind="ExternalInput")
    wp_d = nc.dram_tensor("wp", [GC, C], F32R, kind="ExternalInput")
    out_d = nc.dram_tensor("out", [T, C], BF16, kind="ExternalOutput")

    with ExitStack() as ctx:
        tc = ctx.enter_context(tile.TileContext(nc))
        const = ctx.enter_context(tc.tile_pool(name="const", bufs=1))
        big = ctx.enter_context(tc.tile_pool(name="big", bufs=1))
        xtp = ctx.enter_context(tc.tile_pool(name="xtp", bufs=2))
        qtp = ctx.enter_context(tc.tile_pool(name="qtp", bufs=2))
        ytp = ctx.enter_context(tc.tile_pool(name="ytp", bufs=3))
        ptp = ctx.enter_context(tc.tile_pool(name="ptp", bufs=6))
        rp = ctx.enter_context(tc.tile_pool(name="rp", bufs=2))
        obp = ctx.enter_context(tc.tile_pool(name="obp", bufs=2))
        psS = ctx.enter_context(tc.tile_pool(name="psS", bufs=2, space="PSUM"))
        psY = ctx.enter_context(tc.tile_pool(name="psY", bufs=2, space="PSUM"))
        psB = ctx.enter_context(tc.tile_pool(name="psB", bufs=2, space="PSUM"))

        # causal mask, replicated for the two heads of an exp pair
        # (built after the startup DMAs are issued -- see q == 0 below --
        # so the gpsimd queue isn't busy ahead of the SWDGE x fetch)
        mask2 = const.tile([128, 2, KBLK], BF16)
        bqk_sb = const.tile([128, 2, 3], F32)

        wa = big.tile([128, CT, 3 * GC], BF16)
        wp = big.tile([128, NP, C], F32R)
        # K for heads 0-3 in fp8 d-half-pair layout: [32h+d%32, d//32, tok]
        kAB = big.tile([128, 2, T], F8)
        # K for heads 4,5: bf16 d-major
        kC = big.tile([128, T], BF16)
        # V interleaved with ones columns: even head h -> [V_h | 1],
        # odd head h -> [1 | V_h]; a single M=128 matmul then yields
        # y^T on one 64-partition half and the exp row-sums on the other.
        vs = big.tile([128, NK, HG, 2 * D], BF16)

        xt_r = xt_d[:, :].rearrange("(ct r) t -> ct r t", r=128)
        wa_r = wa_d[:, :].rearrange("(ct r) j -> ct r j", r=128)
        wp_r = wp_d[:, :].rearrange("(p r) e -> p r e", r=128)

        def dma_xtq(xtq, qs):
            for ct in range(CT):
                nc.sync.dma_start(out=xtq[:, ct, :],
                                  in_=xt_r[ct][:, qs:qs + QBLK])

        def qk_group(xtq, qAB, qC, qs, which, t):
            """One Q-or-K 128-channel psum tile: 6 matmuls + eviction
            (fp8 pair layout for tiles 0-1, bf16 d-major for tile 2),
            split into two half-closures so the interleaver can place
            them at sub-group granularity."""
            cell = {}

            def half(lo, hi, evict):
                def mms():
                    _mark(nc, f"qk.w{which}.t{t}")
                    if lo == 0:
                        cell["pqk"] = psB.tile([128, QBLK], F32, tag="b",
                                               name="pqk")
                    pqk = cell["pqk"]
                    for ct in range(lo, hi):
                        nc.tensor.matmul(
                            pqk,
                            lhsT=wa[:, ct, which * GC + t * 128:
                                           which * GC + (t + 1) * 128],
                            rhs=xtq[:, ct, :],
                            start=(ct == 0), stop=(ct == CT - 1))
                    if evict:
                        sc = bqk_sb[:, which, t:t + 1]
                        if t < 2:
                            dest = qAB[:, t, :] if which == 0 \
                                else kAB[:, t, qs:qs + QBLK]
                        else:
                            dest = qC if which == 0 \
                                else kC[:, qs:qs + QBLK]
                        nc.vector.tensor_scalar_add(dest, pqk, sc)
                return mms
            return [half(0, 3, False), half(3, CT, True)]

        def v_group(xtq, k_i, kl):
            def mms():
                _mark(nc, f"v.k{k_i}")
                pv = psB.tile([128, QBLK], F32, tag="b", name="pv")
                for ct in range(CT):
                    nc.tensor.matmul(
                        pv[:, 0:GC],
                        lhsT=xtq[:, ct, kl * KBLK:(kl + 1) * KBLK],
                        rhs=wa[:, ct, 2 * GC:3 * GC],
                        start=(ct == 0), stop=(ct == CT - 1))
                pv3 = pv[:, 0:GC].rearrange("r (a b d) -> r a b d", b=2, d=D)
                vsv = vs[:, k_i].rearrange("r (a b) c -> r a b c", b=2)
                vso = vs[:, k_i].rearrange("r (a b) c -> r a (b c)", b=2)
                # ones occupy the middle 128 cols of each pair's 256 block;
                # even head V -> cols 0:64, odd head V -> cols 64:128 of its
                # own block
                nc.gpsimd.memset(vso[:, :, D:3 * D], 1.0)
                nc.vector.tensor_copy(vsv[:, :, 0, 0:D], pv3[:, :, 0, :])
                nc.vector.tensor_copy(vsv[:, :, 1, D:2 * D], pv3[:, :, 1, :])
            return mms

        def proj_group(yt, qs, tt, split_dma=False):
            def mms():
                _mark(nc, f"proj.tt{tt}")
                t0 = qs + tt * KBLK
                ob = obp.tile([128, C], BF16, tag="ob", name="ob")
                for ec in range(2):
                    po = psB.tile([128, QBLK], F32, tag="b", name="po")
                    for j in range(NP):
                        nc.tensor.matmul(
                            po[:, 0:GC],
                            lhsT=yt[:, j, tt * KBLK:(tt + 1) * KBLK],
                            rhs=wp[:, j, ec * GC:(ec + 1) * GC],
                            start=(j == 0), stop=(j == NP - 1))
                    if split_dma and ec == 1:
                        # tail: Act is idle; do the second eviction there so
                        # both halves evict in parallel
                        nc.scalar.copy(ob[:, ec * GC:(ec + 1) * GC],
                                       po[:, 0:GC])
                    else:
                        nc.vector.tensor_copy(ob[:, ec * GC:(ec + 1) * GC],
                                              po[:, 0:GC])
                    if split_dma:
                        # tail: fire each half as soon as it is evicted
                        q_eng = nc.sync if ec == 0 else nc.scalar
                        q_eng.dma_start(
                            out=out_d[t0:t0 + KBLK, ec * GC:(ec + 1) * GC],
                            in_=ob[:, ec * GC:(ec + 1) * GC])
                if not split_dma:
                    q_eng = nc.sync if tt % 2 == 0 else nc.scalar
                    q_eng.dma_start(out=out_d[t0:t0 + KBLK, :], in_=ob)
            return mms

        proj_queue = []         # deferred c_proj groups of earlier q-tiles
        carry = []              # fillers deferred to the next tile
        for q in range(NQ):
            qs = q * QBLK
            if q == 0:
                # startup: the QKV matmul for contraction tile ct needs the
                # (wa-qk[ct], xtq[ct]) pair, so stream those as interleaved
                # pairs on the two HWDGE queues; everything else follows.
                xtq = xtp.tile([128, CT, QBLK], BF16, tag="xtq", name="xtq")
                for ct in range(CT):
                    nc.scalar.dma_start(out=wa[:, ct, 0:2 * GC],
                                        in_=wa_r[ct][:, 0:2 * GC])
                    nc.gpsimd.dma_start(out=xtq[:, ct, :],
                                        in_=xt_r[ct][:, 0:QBLK])
                    if ct == 0:
                        nc.sync.dma_start(out=bqk_sb, in_=bqk_d[:, :, :])
                for ct in range(CT):
                    nc.scalar.dma_start(out=wa[:, ct, 2 * GC:3 * GC],
                                        in_=wa_r[ct][:, 2 * GC:3 * GC])
                pref_xtq = xtp.tile([128, CT, QBLK], BF16, tag="xtq",
                                    name="xtq")
                for ct in range(CT):
                    nc.gpsimd.dma_start(out=pref_xtq[:, ct, :],
                                        in_=xt_r[ct][:, QBLK:2 * QBLK])
                for pp in range(NP):
                    nc.scalar.dma_start(out=wp[:, pp, :], in_=wp_r[pp])
                make_upper_triangular(nc, mask2[:, 0, :], val=1.0, diag=True)
                make_upper_triangular(nc, mask2[:, 1, :], val=1.0, diag=True)
                qAB = qtp.tile([128, 2, QBLK], F8, tag="qAB", name="qAB")
                qC = qtp.tile([128, QBLK], BF16, tag="qC", name="qC")
                for which in (0, 1):
                    for t in range(3):
                        for h in qk_group(xtq, qAB, qC, qs, which, t):
                            h()
                for k_i in range(4):
                    v_group(xtq, k_i, k_i)()

            # fillers interleaved into this q-tile's attention stream.  The
            # early tiles are PE-rich (small attention) and the late ones
            # Act-bound, so PE-side work is skewed late: the last tile gets
            # the K-projection of its own QKV plus two tiles' worth of
            # c_proj from the deferred queue.
            fillers = list(carry)
            n_early = len(fillers)   # carried items are dependency-critical
            carry = []
            if q + 1 < NQ:
                nqs = qs + QBLK
                if q == 0:
                    nxtq = pref_xtq
                else:
                    nxtq = xtp.tile([128, CT, QBLK], BF16, tag="xtq",
                                    name="xtq")
                    dma_xtq(nxtq, nqs)
                nqAB = qtp.tile([128, 2, QBLK], F8, tag="qAB", name="qAB")
                nqC = qtp.tile([128, QBLK], BF16, tag="qC", name="qC")
                for t in range(3):
                    fillers += qk_group(nxtq, nqAB, nqC, nqs, 0, t)
                kq = [h for t in range(3)
                      for h in qk_group(nxtq, nqAB, nqC, nqs, 1, t)]
                if q + 1 == NQ - 1:
                    carry += kq      # K of the last tile: emit during it
                else:
                    fillers += kq
                for kl in range(4):
                    fillers.append(v_group(nxtq, 4 * (q + 1) + kl, kl))
            # deferred c_proj: none before att(2); proj(0) at att(2);
            # the rest at att(3)
            if q == NQ - 1:
                fillers += proj_queue
                proj_queue = []

            nkt = (q + 1) * (QBLK // KBLK)
            n_slots = NP * nkt
            yt = ytp.tile([128, NP, QBLK], F32R, tag="yt", name="yt")
            slot = 0
            emitted = 0

            for p in range(NP):
                ya = psY.tile([128, QBLK], F32, tag="y", name="ya")
                yb = psY.tile([128, QBLK], F32, tag="y", name="yb")
                def emit_S(k_i, p=p):
                    """S matmuls for (p, k_i); returns state for emit_exp."""
                    _mark(nc, f"att.q{q}.p{p}.k{k_i}")
                    m = k_i - 4 * q
                    col0 = max(m, 0) * KBLK
                    st2 = psS.tile([128, 2, QBLK], F32, tag="st", name="st2")
                    for s in range(2):
                        h = 2 * p + s
                        if h < 4:
                            base = 32 * h
                            nc.tensor.matmul(
                                st2[:, s, col0:QBLK],
                                lhsT=kAB[base:base + 32, :,
                                         k_i * KBLK:(k_i + 1) * KBLK],
                                rhs=qAB[base:base + 32, :, col0:QBLK],
                                start=True, stop=True, perf_mode=DR,
                                tile_position=(base, 0))
                        else:
                            hb = 64 * (h - 4)
                            nc.tensor.matmul(
                                st2[:, s, col0:QBLK],
                                lhsT=kC[hb:hb + 64,
                                        k_i * KBLK:(k_i + 1) * KBLK],
                                rhs=qC[hb:hb + 64, col0:QBLK],
                                start=True, stop=True)
                    return (k_i, m, col0, st2)

                # S runs one iteration ahead of exp/PV so a filler emitted
                # between iterations never delays the exp the Act engine is
                # about to run (engines execute their streams in order).
                pend = None
                sq = emit_S(0)
                for k_i in range(nkt):
                    if k_i + 1 < nkt:
                        nxt = emit_S(k_i + 1)
                    _, m, col0, st2 = sq
                    pt2 = ptp.tile([128, 2, QBLK], BF16, tag="pt", name="pt2")
                    nc.scalar.activation(pt2[:, :, col0:QBLK],
                                         st2[:, :, col0:QBLK],
                                         AF.Exp, scale=0.125)
                    if m >= 0:
                        seg = pt2[:, :, col0:col0 + KBLK]
                        nc.vector.tensor_mul(seg, seg, mask2)
                    if pend is not None:
                        pend()
                    first = (k_i == 0)
                    last = (k_i == nkt - 1)

                    def make_pv(pt2=pt2, p=p, col0=col0, first=first,
                                last=last, k_i=k_i, ya=ya, yb=yb):
                        def pv():
                            for s in range(2):
                                yy = ya if s == 0 else yb
                                nc.tensor.matmul(
                                    yy[:, col0:QBLK],
                                    lhsT=vs[:, k_i, 2 * p + s, :],
                                    rhs=pt2[:, s, col0:QBLK],
                                    start=first, stop=last,
                                    skip_group_check=True)
                        return pv
                    pend = make_pv()
                    if k_i + 1 < nkt:
                        sq = nxt
                    # interleave cross-phase matmul groups (front-loaded so
                    # dependency-critical groups land before their readers)
                    slot += 1
                    want = min(len(fillers),
                               max((slot * len(fillers)) // n_slots,
                                   min(slot, n_early)))
                    while emitted < want:
                        fillers[emitted]()
                        emitted += 1
                pend()
                _mark(nc, f"norm.q{q}.p{p}")
                # normalize: y^T / r.  Evict psum, partition-shift r via
                # gpsimd, single-pass reciprocal, two multiplies.  For the
                # final tile's last pair this is the tail critical path, so
                # run it in 128-column slices that unblock proj(tt) early.
                ya_sb = rp.tile([128, QBLK], F32, tag="ya", name="ya_sb")
                yb_sb = rp.tile([128, QBLK], F32, tag="yb", name="yb_sb")
                rsh = rp.tile([128, QBLK], F32, tag="rsh", name="rsh")
                rec = rp.tile([128, QBLK], F32, tag="rec", name="rec")
                tail = (q == NQ - 1 and p == NP - 1)
                n_sl = 4 if tail else 1
                for sl in range(n_sl):
                    cs = slice(sl * (QBLK // n_sl), (sl + 1) * (QBLK // n_sl))
                    nc.vector.tensor_copy(ya_sb[:, cs], ya[:, cs])
                    if tail:
                        # Act is idle once the last exp retires; use it for
                        # the second eviction to shorten the tail chain
                        nc.scalar.copy(yb_sb[:, cs], yb[:, cs])
                    else:
                        nc.vector.tensor_copy(yb_sb[:, cs], yb[:, cs])
                    nc.gpsimd.tensor_copy(rsh[0:64, cs], ya_sb[64:128, cs])
                    nc.gpsimd.tensor_copy(rsh[64:128, cs], yb_sb[0:64, cs])
                    nc.vector.reciprocal_approx_fast(rec[:, cs], rsh[:, cs])
                    nc.vector.tensor_mul(yt[0:64, p, cs], ya_sb[0:64, cs],
                                         rec[0:64, cs])
                    nc.vector.tensor_mul(yt[64:128, p, cs],
                                         yb_sb[64:128, cs],
                                         rec[64:128, cs])
            while emitted < len(fillers):
                fillers[emitted]()
                emitted += 1
            proj_queue += [proj_group(yt, qs, tt, split_dma=(q == NQ - 1))
                           for tt in range(QBLK // KBLK)]
            if q + 1 < NQ:
                xtq, qAB, qC = nxtq, nqAB, nqC
        for g in proj_queue:
            g()
    nc.compile()
    return nc


def make_in_map(x_b, w_attn, b_attn, w_proj, g):
    """Per-core input arrays for batch slice x_b and head-group g."""
    sl = slice(g * GC, (g + 1) * GC)
    perm = _qk_perm()
    wq = w_attn[:, 0 * C:1 * C][:, sl][:, perm]
    wk = w_attn[:, 1 * C:2 * C][:, sl][:, perm]
    wv = w_attn[:, 2 * C:3 * C][:, sl]
    bq = b_attn[0 * C:1 * C][sl][perm]
    bk = b_attn[1 * C:2 * C][sl][perm]
    # [128, 2, 3]: per-partition bias for the 3 Q/K psum tiles
    bqk = np.ascontiguousarray(
        np.stack([bq, bk]).reshape(2, 3, 128).transpose(2, 0, 1))
    import ml_dtypes
    return {
        "xt": np.ascontiguousarray(x_b.T).astype(ml_dtypes.bfloat16),
        "wa": np.ascontiguousarray(
            np.concatenate([wq, wk, wv], axis=1)).astype(ml_dtypes.bfloat16),
        "bqk": bqk,
        "wp": np.ascontiguousarray(w_proj[sl, :]),
    }


_NC_CACHE = {}


def _get_nc(T):
    if T not in _NC_CACHE:
        _NC_CACHE[T] = build_nc(T)
    return _NC_CACHE[T]


def kernel(x, w_attn, b_attn, w_proj, b_proj, _trace=False):
    from concourse.bass_utils import run_bass_kernel_spmd

    x = np.asarray(x, dtype=np.float32)
    w_attn = np.asarray(w_attn, dtype=np.float32)
    b_attn = np.asarray(b_attn, dtype=np.float32)
    w_proj = np.asarray(w_proj, dtype=np.float32)
    b_proj = np.asarray(b_proj, dtype=np.float32)
    B, T, _ = x.shape

    nc = _get_nc(T)
    in_maps = []
    for b in range(B):
        for g in range(2):
            in_maps.append(make_in_map(x[b], w_attn, b_attn, w_proj, g))
    res = run_bass_kernel_spmd(nc, in_maps, core_ids=list(range(2 * B)),
                               trace=_trace)
    outs = [np.asarray(r["out"], dtype=np.float32) for r in res.results]
    # softmax rows sum to 1, so the V-bias contribution is exactly
    # bv @ w_proj added to every token (not computed on device).
    bias_row = b_proj + b_attn[2 * C:3 * C] @ w_proj
    out = np.empty((B, T, C), dtype=np.float32)
    for b in range(B):
        out[b] = outs[2 * b] + outs[2 * b + 1] + bias_row[None, :]
    if _trace:
        kernel.last_result = res
    return out
